# revision 1
# baseline (speedup 1.0000x reference)
"""GQA causal attention (llama3-style RoPE) on 8 TRN2 NeuronCores.

Sharding: tensor-parallel over heads. Core c gets q-heads 4c..4c+3 and
kv-head c (GQA groups intact), plus the matching row-block of wo.T.
Each core computes a full [S, D] partial of the output projection;
the host sums the 8 partials (the "all-reduce" of the row-sharded wo).

Per-core pipeline (all layouts chosen so no on-device transpose of x/q/k
is ever needed):
  qkvT[col, s]  = wqkvT.T @ xT          (weights stationary, xT streaming)
  RoPE on qT/kT (de-interleaved pair layout via host-permuted weight rows)
  sT[sk, sq]    = kT.T @ qT             (K=64)
  eT            = exp(sT/8) * causal_mask
  avT[hd+1, sq] = v_aug.T @ eT          (v augmented with a ones row ->
                                         numerator and denominator in one
                                         accumulation)
  yT            = avT[0:64] * bcast(1/avT[64])
  out[sq, d]    = yT.T @ woT            (partial; host sums over cores)
"""

import sys

for _p in ("/opt/trn_rl_repo", "/root/.axon_site/_ro/trn_rl_repo"):
    if _p not in sys.path:
        sys.path.insert(0, _p)

import numpy as np
import ml_dtypes

import concourse.bass as bass
import concourse.bacc as bacc
import concourse.mybir as mybir
import concourse.tile as tile

BF16 = ml_dtypes.bfloat16

S = 2048
D = 2048
HD = 64
NH = 32
NKV = 8
NCORES = 8
QH = NH // NCORES            # 4 local q heads
QCOLS = QH * HD              # 256
KVCOLS = 2 * HD              # 128 (k and v, one kv head)
P = 128                      # partitions
NK = D // P                  # 16 contraction tiles
NSQ = S // P                 # 16 seq tiles of 128
NCH = 4                      # seq chunks of 512
CH = 512

_CACHE = {}


def _build():
    mm_dt = mybir.dt.bfloat16
    f16 = mybir.dt.float16
    f32 = mybir.dt.float32

    nc = bacc.Bacc()
    xt_d = nc.dram_tensor("xt", [D, S], mm_dt, kind="ExternalInput")
    wqkvt_d = nc.dram_tensor("wqkvt", [D, QCOLS + KVCOLS], mm_dt, kind="ExternalInput")
    wot_d = nc.dram_tensor("wot", [QCOLS, D], mm_dt, kind="ExternalInput")
    cos_d = nc.dram_tensor("cos64", [HD, S], f16, kind="ExternalInput")
    swap_d = nc.dram_tensor("swap64", [HD, S], f16, kind="ExternalInput")
    masks_d = nc.dram_tensor("masks", [P, P], mm_dt, kind="ExternalInput")
    ones_d = nc.dram_tensor("ones64", [1, HD], f32, kind="ExternalInput")
    ident_d = nc.dram_tensor("ident", [HD, HD], mm_dt, kind="ExternalInput")
    out_d = nc.dram_tensor("out", [S, D], f32, kind="ExternalOutput")

    with tile.TileContext(nc) as tc:
        with (
            tc.tile_pool(name="const", bufs=1) as cpool,
            tc.tile_pool(name="xt", bufs=NK) as xpool,
            tc.tile_pool(name="wq", bufs=NK) as wpool,
            tc.tile_pool(name="big", bufs=1) as bigpool,
            tc.tile_pool(name="vaug", bufs=NSQ) as vpool,
            tc.tile_pool(name="et", bufs=20) as epool,
            tc.tile_pool(name="tmp", bufs=3) as tpool,
            tc.tile_pool(name="ps_a", bufs=2, space="PSUM") as ps_a,
            tc.tile_pool(name="ps_s", bufs=2, space="PSUM") as ps_s,
            tc.tile_pool(name="ps_av", bufs=2, space="PSUM") as ps_av,
        ):
            # ---- constants / weights in ----
            # small tables via SWDGE (gpsimd); bulk via the two HWDGE
            # queues (SP + ACT) in parallel
            cos_sb = cpool.tile([HD, S], f16, tag="cos")
            swap_sb = cpool.tile([HD, S], f16, tag="swap")
            masks_sb = cpool.tile([P, P], mm_dt, tag="masks")
            ones_sb = cpool.tile([1, HD], f32, tag="ones")
            ident_sb = cpool.tile([HD, HD], mm_dt, tag="ident")
            zbias = cpool.tile([P, 1], f32, tag="zbias")
            nc.gpsimd.memset(zbias[:], 0.0)
            nc.gpsimd.dma_start(cos_sb[:], cos_d[:])
            nc.gpsimd.dma_start(swap_sb[:], swap_d[:])
            nc.gpsimd.dma_start(masks_sb[:], masks_d[:])
            nc.gpsimd.dma_start(ones_sb[:], ones_d[:])
            nc.gpsimd.dma_start(ident_sb[:], ident_d[:])

            hwdge = [nc.sync, nc.scalar]
            xt_sb = []
            wq_sb = []
            for k in range(NK):
                w = wpool.tile([P, QCOLS + KVCOLS], mm_dt, tag="wq", name=f"wqkv{k}")
                hwdge[k % 2].dma_start(w[:], wqkvt_d[k * P : (k + 1) * P, :])
                wq_sb.append(w)
            for k in range(NK):
                xt_sb.append(xpool.tile([P, S], mm_dt, tag="xt", name=f"xt{k}"))
            for q in range(NCH):
                qs = slice(q * CH, (q + 1) * CH)
                for k in range(NK):
                    eng = [nc.sync, nc.scalar, nc.gpsimd][k % 3] if q == 0 else hwdge[k % 2]
                    eng.dma_start(xt_sb[k][:, qs], xt_d[k * P : (k + 1) * P, qs])

            wot_sb = []
            for k in range(2):
                t = cpool.tile([P, D], mm_dt, tag=f"wot{k}", name=f"wot{k}")
                nc.gpsimd.dma_start(t[:], wot_d[k * P : (k + 1) * P, :])
                wot_sb.append(t)

            qt_sb = [bigpool.tile([P, S], f16, tag=f"qt{m}", name=f"qt{m}") for m in range(QH)]
            kt_sb = bigpool.tile([P, S], f16, tag="kt")
            vt_sb = bigpool.tile([HD, S], mm_dt, tag="vt")
            yt_sb = [bigpool.tile([P, S], mm_dt, tag=f"yt{m}", name=f"yt{m}") for m in range(2)]

            # ---- projections: qkvT[col, sq] accumulated over d ----
            # Mtile order: kv first so SDPA can start as soon as q is ready.
            # Mtile 2: [kT; vT] | Mtile 0: q heads 0,1 | Mtile 1: q heads 2,3
            def rope(dst, ps, chunk):
                # dst[:, chunk] = RoPE(ps) for one 64-row de-interleaved head.
                # Drain psum to f16 SBUF once so the elementwise ops run in
                # the DVE 2-byte SBUF fast mode.
                qr = tpool.tile([HD, CH], f16, tag="rope_qr", name="rope_qr")
                nc.vector.tensor_copy(qr[:], ps[:])
                t2 = tpool.tile([HD, CH], f16, tag="rope_t2", name="rope_t2")
                nc.vector.tensor_mul(t2[0:32, :], qr[32:64, :], swap_sb[32:64, chunk])
                nc.vector.tensor_mul(t2[32:64, :], qr[0:32, :], swap_sb[0:32, chunk])
                nc.vector.tensor_mul(dst[:, chunk], qr[:], cos_sb[:, chunk])
                nc.vector.tensor_add(dst[:, chunk], dst[:, chunk], t2[:])

            vaug_sb = [None] * NSQ

            def vtrans(jlist):
                with nc.named_scope("vtrans"):
                    for i in jlist:
                        pt = ps_av.tile([P, HD], mm_dt, tag="av", name="ps_vt")
                        nc.tensor.transpose(
                            pt[:], vt_sb[:, i * P : (i + 1) * P], ident_sb[:]
                        )
                        va = vpool.tile([P, HD + 1], mm_dt, tag="vaug", name=f"vaug{i}")
                        nc.vector.tensor_copy(va[:, 0:HD], pt[:])
                        nc.gpsimd.memset(va[:, HD : HD + 1], 1.0)
                        vaug_sb[i] = va

            for m in (2, 0, 1):
                if m == 1:
                    vtrans(range(NSQ))
                for j in range(NCH):
                    chunk = slice(j * CH, (j + 1) * CH)
                    ps = ps_a.tile([P, CH], f32, tag="proj", name="ps_proj")
                    with nc.named_scope("proj"):
                        for k in range(NK):
                            nc.tensor.matmul(
                                ps[:],
                                wq_sb[k][:, m * P : (m + 1) * P],
                                xt_sb[k][:, chunk],
                                start=(k == 0),
                                stop=(k == NK - 1),
                            )
                    with nc.named_scope("rope"):
                        if m < 2:
                            rope(qt_sb[2 * m][0:HD, :], ps[0:HD, :], chunk)
                            rope(qt_sb[2 * m + 1][0:HD, :], ps[HD:P, :], chunk)
                            for hh in (2 * m, 2 * m + 1):
                                nc.gpsimd.dma_start(
                                    qt_sb[hh][HD:P, chunk], qt_sb[hh][0:HD, chunk]
                                )
                        else:
                            rope(kt_sb[0:HD, :], ps[0:HD, :], chunk)
                            nc.gpsimd.dma_start(kt_sb[HD:P, chunk], kt_sb[0:HD, chunk])
                            nc.vector.tensor_copy(vt_sb[:, chunk], ps[HD:P, :])

            # ---- SDPA per (head, sq-chunk), causal ----
            # sk-tile pairs run concurrently in the PE array via row groups
            # (K=64): pair element 0 in rows 0-63, element 1 in rows 64-127.
            # Each pair writes one [128, 1024] 2-bank psum tile so the exp
            # over both halves is a single ACT op. Boundary tiles
            # (o = i-4j >= 0) only compute/exp columns [128*o:512); the
            # first 128 of those get the triangular mask.
            for j in range(NCH):
                for h in range(QH):
                    qrow = (h % 2) * HD
                    chunk = slice(j * CH, (j + 1) * CH)
                    nlive = 4 * j + 4  # sk tiles 0..4j+3 are causal-live
                    offs = [max(0, (i - 4 * j)) * P for i in range(nlive)]
                    ets = []
                    with nc.named_scope("scores"):
                        for i in range(0, nlive, 2):
                            ps2 = ps_s.tile([P, 2 * CH], f32, tag="sc", name="ps_sc")
                            for u in range(2):
                                off = offs[i + u]
                                rg = slice(u * HD, (u + 1) * HD)
                                nc.tensor.matmul(
                                    ps2[:, u * CH + off : (u + 1) * CH],
                                    kt_sb[rg, (i + u) * P : (i + u + 1) * P],
                                    qt_sb[h][rg, j * CH + off : (j + 1) * CH],
                                    start=True,
                                    stop=True,
                                )
                            et2 = epool.tile([P, 2 * CH], mm_dt, tag="et", name="et")
                            with nc.named_scope("exp"):
                                if offs[i] == 0 and offs[i + 1] == 0:
                                    nc.scalar.activation(
                                        et2[:],
                                        ps2[:],
                                        mybir.ActivationFunctionType.Exp,
                                        bias=zbias[:],
                                        scale=0.125,
                                    )
                                else:
                                    for u in range(2):
                                        off = offs[i + u]
                                        nc.scalar.activation(
                                            et2[:, u * CH + off : (u + 1) * CH],
                                            ps2[:, u * CH + off : (u + 1) * CH],
                                            mybir.ActivationFunctionType.Exp,
                                            bias=zbias[:],
                                            scale=0.125,
                                        )
                            for u in range(2):
                                if i + u >= nlive - 4:  # boundary tile
                                    off = u * CH + offs[i + u]
                                    with nc.named_scope("mask"):
                                        nc.vector.tensor_mul(
                                            et2[:, off : off + P],
                                            et2[:, off : off + P],
                                            masks_sb[:],
                                        )
                            ets.append(et2)
                    pav = ps_av.tile([HD + 1, CH], f32, tag="av", name="ps_av")
                    with nc.named_scope("av"):
                        for i in range(nlive):
                            off = offs[i]
                            nc.tensor.matmul(
                                pav[:, off:],
                                vaug_sb[i][:],
                                ets[i // 2][:, (i % 2) * CH + off : (i % 2 + 1) * CH],
                                start=(i == 0),
                                stop=(i == nlive - 1),
                            )
                    # normalize: yT = avT[0:64] / avT[64]
                    with nc.named_scope("norm"):
                        recip = tpool.tile([1, CH], f32, tag="recip", name="recip")
                        nc.vector.reciprocal(recip[:], pav[HD : HD + 1, :])
                        bc = tpool.tile([HD, CH], f32, tag="bc", name="bc")
                        nc.gpsimd.partition_broadcast(bc[:], recip[:])
                        nc.vector.tensor_mul(
                            yt_sb[h // 2][qrow : qrow + HD, chunk], pav[0:HD, :], bc[:]
                        )

            # ---- output projection partial: out[sq, d] ----
            for sm in range(NSQ):
                srow = slice(sm * P, (sm + 1) * P)
                for dcJ in range(NCH):
                    dch = slice(dcJ * CH, (dcJ + 1) * CH)
                    pw = ps_a.tile([P, CH], f32, tag="proj", name="ps_wo")
                    with nc.named_scope("wo"):
                        for k in range(2):
                            nc.tensor.matmul(
                                pw[:],
                                yt_sb[k][:, srow],
                                wot_sb[k][:, dch],
                                start=(k == 0),
                                stop=(k == 1),
                            )
                    ot = tpool.tile([P, CH], f32, tag="ot", name="ot")
                    with nc.named_scope("outdma"):
                        nc.vector.tensor_copy(ot[:], pw[:])
                        if sm >= 14:
                            half = CH // 2
                            d0 = dcJ * CH
                            nc.sync.dma_start(out_d[srow, d0 : d0 + half], ot[:, 0:half])
                            nc.gpsimd.dma_start(
                                out_d[srow, d0 + half : d0 + CH], ot[:, half:CH]
                            )
                        else:
                            odma = [nc.sync, nc.gpsimd][(sm * NCH + dcJ) % 2]
                            odma.dma_start(out_d[srow, dch], ot[:])

    nc.finalize()
    return nc


def _host_inputs(x, freqs_cos, freqs_sin, wq, wk, wv, wo):
    """Build the 8 per-core input maps (all host-side preprocessing)."""
    x = np.asarray(x, np.float32)
    cos = np.asarray(freqs_cos, np.float32)  # [S, 32]
    sin = np.asarray(freqs_sin, np.float32)
    wq = np.asarray(wq, np.float32)
    wk = np.asarray(wk, np.float32)
    wv = np.asarray(wv, np.float32)
    wo = np.asarray(wo, np.float32)

    perm = np.concatenate([np.arange(0, HD, 2), np.arange(1, HD, 2)])  # de-interleave

    xt = np.ascontiguousarray(x[0].T).astype(BF16)

    # cos64[d, t] = cos[t, d % 32]; swap64 rows 0:32 = +sin (imag-out),
    # rows 32:64 = -sin (real-out) so both tensor_mul inputs share a base
    # partition (walrus SB-SB constraint)
    cos64 = np.empty((HD, S), np.float16)
    swap64 = np.empty((HD, S), np.float16)
    for dd in range(HD):
        i = dd % 32
        cos64[dd] = cos[:, i]
        swap64[dd] = sin[:, i] if dd < 32 else -sin[:, i]

    pp = np.arange(P)[:, None]
    ff = np.arange(P)[None, :]
    masks = (pp <= ff).astype(np.float32).astype(BF16)

    ones64 = np.ones((1, HD), np.float32)
    ident = np.eye(HD, dtype=np.float32).astype(BF16)

    in_maps = []
    for c in range(NCORES):
        wq_c = wq[c * QCOLS : (c + 1) * QCOLS].reshape(QH, HD, D)[:, perm, :].reshape(
            QCOLS, D
        )
        wk_c = wk[c * HD : (c + 1) * HD][perm, :]
        wv_c = wv[c * HD : (c + 1) * HD]
        wqkvt = np.ascontiguousarray(
            np.concatenate([wq_c, wk_c, wv_c], axis=0).T
        ).astype(BF16)
        wot = np.ascontiguousarray(wo[:, c * QCOLS : (c + 1) * QCOLS].T).astype(BF16)
        in_maps.append(
            {
                "xt": xt,
                "wqkvt": wqkvt,
                "wot": wot,
                "cos64": cos64,
                "swap64": swap64,
                "masks": masks,
                "ones64": ones64,
                "ident": ident,
            }
        )
    return in_maps


def kernel(x, freqs_cos, freqs_sin, wq, wk, wv, wo):
    from concourse.bass_utils import run_bass_kernel_spmd

    if "nc" not in _CACHE:
        _CACHE["nc"] = _build()
    nc = _CACHE["nc"]
    in_maps = _host_inputs(x, freqs_cos, freqs_sin, wq, wk, wv, wo)
    res = run_bass_kernel_spmd(nc, in_maps, core_ids=list(range(NCORES)))
    out = np.zeros((S, D), np.float64)
    for r in res.results:
        out += r["out"].astype(np.float64)
    return out.astype(np.float32).reshape(1, S, D)



# revision 25
# speedup vs baseline: 1.2789x; 1.2789x over previous
"""GQA causal attention (llama3-style RoPE) on 8 TRN2 NeuronCores.

Sharding: tensor-parallel over heads. Core c gets q-heads 4c..4c+3 and
kv-head c (GQA groups intact), plus the matching row-block of wo.T.
Each core computes a full [S, D] partial of the output projection;
the host sums the 8 partials (the "all-reduce" of the row-sharded wo).

v3 per-core pipeline, fully chunk-interleaved so ACT (exp) starts ~10us
into the kernel and PE never waits long:
  for each 512-col seq chunk j:
    qkvT[col, js] = wqkvT.T @ xT    fp8e4m3 hi/lo 3-term DoubleRow
                                    (weights host-scaled x256; descale
                                    folded into the exp scale / v drain)
    RoPE on qT/kT                   two q heads stacked per [128, 512]
    v -> vaug[sk, hd|1]             PE transpose, Pool drain
    sT[sk, 2, sq] = kT.T @ qT       K=64 row-groups: head-even rows
                                    0:64, head-odd rows 64:128 (kt dup)
    eT = exp(sT/(8*WS^2) - 3)       one ACT op per [128, 2, 512-off]
    av[sq, 2*(hd|den)] = et.T @ vaug   et slabs stationary (LdWeights
                                    free), 65-wide moving operand
    y2 = av/den                     DVE tensor_scalar divide
    yt = dma_transpose(y2)          SP queue, xbar 14ns/tile
    out[sq, d] = yT.T @ woT         bf16 partial out; host sums cores
"""

import sys

for _p in ("/opt/trn_rl_repo", "/root/.axon_site/_ro/trn_rl_repo"):
    if _p not in sys.path:
        sys.path.insert(0, _p)

import numpy as np
import ml_dtypes

import concourse.bacc as bacc
import concourse.mybir as mybir
import concourse.tile as tile

BF16 = ml_dtypes.bfloat16
F8 = ml_dtypes.float8_e4m3

S = 2048
D = 2048
HD = 64
NH = 32
NKV = 8
NCORES = 8
QH = NH // NCORES            # 4 local q heads
QCOLS = QH * HD              # 256
P = 128
NK = D // P                  # 16 contraction tiles
NKP = NK // 2                # 8 DoubleRow pairs
NSQ = S // P                 # 16 seq tiles of 128
NCH = 4                      # seq chunks of 512
CH = 512
WS = 256.0                   # fp8 weight pre-scale
CSH = 3.0                    # exp shift (cancels in softmax ratio)

_CACHE = {}


def _build():
    bf = mybir.dt.bfloat16
    f16 = mybir.dt.float16
    f32 = mybir.dt.float32
    f8 = mybir.dt.float8e4
    DR = mybir.MatmulPerfMode.DoubleRow
    Exp = mybir.ActivationFunctionType.Exp

    nc = bacc.Bacc()
    xhi_d = nc.dram_tensor("xhi", [NKP * P, 2 * S], f8, kind="ExternalInput")
    xlo_d = nc.dram_tensor("xlo", [NKP * P, 2 * S], f8, kind="ExternalInput")
    whi_d = nc.dram_tensor("whi", [NKP * P, 2 * 384], f8, kind="ExternalInput")
    wlo_d = nc.dram_tensor("wlo", [NKP * P, 2 * 384], f8, kind="ExternalInput")
    wot_d = nc.dram_tensor("wot", [QCOLS, D], bf, kind="ExternalInput")
    cos_d = nc.dram_tensor("cos128", [P, S], f16, kind="ExternalInput")
    swap_d = nc.dram_tensor("swap128", [P, S], f16, kind="ExternalInput")
    masks_d = nc.dram_tensor("masks2", [P, 2 * P], bf, kind="ExternalInput")
    ident_d = nc.dram_tensor("ident", [HD, HD], bf, kind="ExternalInput")
    out_d = nc.dram_tensor("out", [S, D], bf, kind="ExternalOutput")

    with tile.TileContext(nc) as tc:
        with (
            tc.tile_pool(name="const", bufs=1) as cpool,
            tc.tile_pool(name="xw", bufs=16) as xwpool,
            tc.tile_pool(name="vaug", bufs=16) as vpool,
            tc.tile_pool(name="et", bufs=33) as epool,
            tc.tile_pool(name="rope", bufs=2) as rpool,
            tc.tile_pool(name="y2", bufs=4) as ypool,
            tc.tile_pool(name="ot", bufs=4) as opool,
            tc.tile_pool(name="ps_a", bufs=2, space="PSUM") as ps_a,
            tc.tile_pool(name="ps_s", bufs=2, space="PSUM") as ps_s,
            tc.tile_pool(name="ps_av", bufs=2, space="PSUM") as ps_av,
        ):
            # ---- constants + weights + x in ------------------------------
            cos_sb = cpool.tile([P, S], f16, tag="cos")
            swap_sb = cpool.tile([P, S], f16, tag="swap")
            masks_sb = cpool.tile([P, 2, P], bf, tag="masks")
            ident_sb = cpool.tile([HD, HD], bf, tag="ident")
            cbias = cpool.tile([P, 1], f32, tag="cbias")
            nc.gpsimd.memset(cbias[:], -CSH)
            nc.gpsimd.dma_start(masks_sb[:], masks_d[:])
            nc.gpsimd.dma_start(ident_sb[:], ident_d[:])

            whi_sb, wlo_sb = [], []
            for kp in range(NKP):
                for lst, dram, nm in ((whi_sb, whi_d, "whi"), (wlo_sb, wlo_d, "wlo")):
                    w = xwpool.tile([P, 2, 384], f8, tag="w", name=f"{nm}{kp}")
                    eng = [nc.sync, nc.scalar][kp % 2]
                    eng.dma_start(w[:], dram[kp * P : (kp + 1) * P, :])
                    lst.append(w)

            xhi_sb, xlo_sb = [], []
            for kp in range(NKP):
                xhi_sb.append(xwpool.tile([P, 2, S], f8, tag="x", name=f"xhi{kp}"))
                xlo_sb.append(xwpool.tile([P, 2, S], f8, tag="x", name=f"xlo{kp}"))
            # x loads in 3 col-groups: [0:512), [512:1024), [1024:2048)
            for gi, (g0, g1) in enumerate(((0, CH), (CH, 2 * CH), (2 * CH, S))):
                for kp in range(NKP):
                    for xi, (sb_t, dram) in enumerate(
                        ((xhi_sb, xhi_d), (xlo_sb, xlo_d))
                    ):
                        src3 = dram[kp * P : (kp + 1) * P, :].rearrange(
                            "p (two s) -> p two s", two=2
                        )
                        eng = (
                            [nc.sync, nc.scalar][(kp + xi) % 2]
                            if gi < 2
                            else [nc.sync, nc.gpsimd][(kp + xi) % 2]
                        )
                        eng.dma_start(sb_t[kp][:, :, g0:g1], src3[:, :, g0:g1])

            nc.scalar.dma_start(cos_sb[:], cos_d[:])
            nc.scalar.dma_start(swap_sb[:], swap_d[:])
            wot_sb = []
            for k in range(2):
                t = cpool.tile([P, D], bf, tag=f"wot{k}", name=f"wot{k}")
                nc.scalar.dma_start(t[:], wot_d[k * P : (k + 1) * P, :])
                wot_sb.append(t)

            qt_sb = [cpool.tile([P, S], f16, tag=f"qt{m}", name=f"qt{m}") for m in range(2)]
            kt_sb = cpool.tile([P, S], f16, tag="kt")
            vt_sb = cpool.tile([HD, S], bf, tag="vt")
            yt_sb = [cpool.tile([P, S], bf, tag=f"yt{m}", name=f"yt{m}") for m in range(2)]
            vaug_sb = [None] * NSQ

            # ---- helpers -------------------------------------------------
            def proj_psum(m, j):
                js = slice(j * CH, (j + 1) * CH)
                ps = ps_a.tile([P, CH], f32, tag="proj", name="ps_proj")
                with nc.named_scope("proj"):
                    n = 0
                    for kp in range(NKP):
                        for wt, xt in (
                            (whi_sb[kp], xhi_sb[kp]),
                            (whi_sb[kp], xlo_sb[kp]),
                            (wlo_sb[kp], xhi_sb[kp]),
                        ):
                            nc.tensor.matmul(
                                ps[:],
                                wt[:, :, m * P : (m + 1) * P],
                                xt[:, :, js],
                                start=(n == 0),
                                stop=(n == 3 * NKP - 1),
                                perf_mode=DR,
                            )
                            n += 1
                return ps

            def rope128(dst, ps, j):
                # two heads stacked: rows 0:64 head-even, 64:128 head-odd
                js = slice(j * CH, (j + 1) * CH)
                with nc.named_scope("rope"):
                    qr = rpool.tile([P, CH], f16, tag="qr", name="qr")
                    nc.vector.tensor_copy(qr[:], ps[:])
                    t2 = rpool.tile([P, CH], f16, tag="t2", name="t2")
                    for b in range(4):
                        d0 = b * 32
                        s0 = (b ^ 1) * 32
                        eng = nc.vector if b < 2 else nc.gpsimd
                        eng.tensor_mul(
                            t2[d0 : d0 + 32, :], qr[s0 : s0 + 32, :], swap_sb[s0 : s0 + 32, js]
                        )
                    nc.vector.tensor_mul(dst[:, js], qr[:], cos_sb[:, js])
                    nc.vector.tensor_add(dst[:, js], dst[:, js], t2[:])

            def rope_kv(ps, j):
                js = slice(j * CH, (j + 1) * CH)
                with nc.named_scope("rope"):
                    qr = rpool.tile([P, CH], f16, tag="qr", name="qr_k")
                    nc.vector.tensor_copy(qr[0:HD, :], ps[0:HD, :])
                    # v drain first: frees the proj psum before the rope muls
                    nc.vector.tensor_scalar_mul(vt_sb[:, js], ps[HD:P, :], 1.0 / WS)
                    t2 = rpool.tile([P, CH], f16, tag="t2", name="t2_k")
                    for b in range(2):
                        d0 = b * 32
                        s0 = (b ^ 1) * 32
                        nc.vector.tensor_mul(
                            t2[d0 : d0 + 32, :], qr[s0 : s0 + 32, :], swap_sb[s0 : s0 + 32, js]
                        )
                    nc.vector.tensor_mul(kt_sb[0:HD, js], qr[0:HD, :], cos_sb[0:HD, js])
                    nc.vector.tensor_add(kt_sb[0:HD, js], kt_sb[0:HD, js], t2[0:HD, :])
                # duplicate k rows for the odd-head row group
                nc.gpsimd.dma_start(kt_sb[HD:P, js], kt_sb[0:HD, js])

            def vtrans(j):
                with nc.named_scope("vtrans"):
                    for i in range(4 * j, 4 * j + 4):
                        pt = ps_av.tile([P, HD], bf, tag="av", name="ps_vt")
                        nc.tensor.transpose(pt[:], vt_sb[:, i * P : (i + 1) * P], ident_sb[:])
                        va = vpool.tile([P, HD + 1], bf, tag="vaug", name=f"vaug{i}")
                        nc.vector.tensor_copy(va[:, 0:HD], pt[:])
                        nc.gpsimd.memset(va[:, HD : HD + 1], 1.0)
                        vaug_sb[i] = va

            all_ets = {}

            def scores_chunk(j, m):
                nlive = 4 * j + 4
                ets = []
                with nc.named_scope("scores"):
                    for i in range(nlive):
                        off = max(0, i - 4 * j) * P
                        ps2 = ps_s.tile([P, 2, CH], f32, tag="sc", name="ps_sc")
                        for u in range(2):
                            rg = slice(u * HD, (u + 1) * HD)
                            nc.tensor.matmul(
                                ps2[:, u, off:],
                                kt_sb[rg, i * P : (i + 1) * P],
                                qt_sb[m][rg, j * CH + off : (j + 1) * CH],
                                start=True,
                                stop=True,
                            )
                        et = epool.tile([P, 2, CH], bf, tag="et", name="et")
                        with nc.named_scope("exp"):
                            nc.scalar.activation(
                                et[:, :, off:],
                                ps2[:, :, off:],
                                Exp,
                                bias=cbias[:],
                                scale=0.125 / (WS * WS),
                            )
                        if i >= 4 * j:
                            with nc.named_scope("mask"):
                                nc.gpsimd.tensor_mul(
                                    et[:, :, off : off + P],
                                    et[:, :, off : off + P],
                                    masks_sb[:],
                                )
                        ets.append(et)
                all_ets[(j, m)] = ets

            def av_chunk(j, m):
                ets = all_ets[(j, m)]
                for r in range(4):
                    t = 4 * j + r
                    pav = ps_av.tile([P, 2 * (HD + 1)], f32, tag="av", name="ps_av")
                    with nc.named_scope("av"):
                        for u in range(2):
                            for i in range(t + 1):
                                nc.tensor.matmul(
                                    pav[:, u * (HD + 1) : (u + 1) * (HD + 1)],
                                    ets[i][:, u, r * P : (r + 1) * P],
                                    vaug_sb[i][:],
                                    start=(i == 0),
                                    stop=(i == t),
                                )
                    with nc.named_scope("norm"):
                        y2 = ypool.tile([P, P], bf, tag="y2", name="y2")
                        rc = ypool.tile([P, 2], f32, tag="rc", name="rc")
                        for u in range(2):
                            c0 = u * (HD + 1)
                            nc.vector.reciprocal(
                                rc[:, u : u + 1], pav[:, c0 + HD : c0 + HD + 1]
                            )
                            nc.vector.tensor_scalar(
                                y2[:, u * HD : (u + 1) * HD],
                                pav[:, c0 : c0 + HD],
                                rc[:, u : u + 1],
                                None,
                                mybir.AluOpType.mult,
                            )
                        nc.sync.dma_start_transpose(
                            yt_sb[m][:, t * P : (t + 1) * P], y2[:]
                        )

            def wo_sm(sm, n):
                srow = slice(sm * P, (sm + 1) * P)
                for half in range(2):
                    ot = opool.tile([P, 2 * CH], bf, tag="ot", name="ot")
                    for q in range(2):
                        dcJ = 2 * half + q
                        dch = slice(dcJ * CH, (dcJ + 1) * CH)
                        pw = ps_a.tile([P, CH], f32, tag="proj", name="ps_wo")
                        with nc.named_scope("wo"):
                            for k in range(2):
                                nc.tensor.matmul(
                                    pw[:],
                                    yt_sb[k][:, srow],
                                    wot_sb[k][:, dch],
                                    start=(k == 0),
                                    stop=(k == 1),
                                )
                        with nc.named_scope("outdma"):
                            # GPSIMD cannot read PSUM: drains go DVE, with
                            # the post-exp last chunk split DVE/ACT
                            if sm >= 12 and (2 * n + q) % 2 == 0:
                                nc.scalar.copy(ot[:, q * CH : (q + 1) * CH], pw[:])
                            else:
                                nc.vector.tensor_copy(ot[:, q * CH : (q + 1) * CH], pw[:])
                    with nc.named_scope("outdma"):
                        nc.sync.dma_start(
                            out_d[srow, half * 2 * CH : (half + 1) * 2 * CH],
                            ot[:],
                        )

            # ---- software-pipelined main loop ----------------------------
            # P(j) proj+rope | S(j) scores+exp | V(j) av+norm | W(j) wo.
            # Interleaved so PE streams P/W work while ACT runs exp ahead.
            def st_p(j):
                rope_kv(proj_psum(2, j), j)
                rope128(qt_sb[0], proj_psum(0, j), j)
                rope128(qt_sb[1], proj_psum(1, j), j)
                vtrans(j)

            def st_s(j):
                scores_chunk(j, 0)
                scores_chunk(j, 1)

            def st_v(j):
                av_chunk(j, 0)
                av_chunk(j, 1)

            def st_w(j):
                for sm in range(4 * j, 4 * j + 4):
                    wo_sm(sm, sm)

            st_p(0)
            st_p(1)
            st_s(0)
            st_p(2)
            st_v(0)
            st_s(1)
            st_p(3)
            st_v(1)
            st_s(2)
            st_v(2)
            scores_chunk(3, 0)
            st_w(0)
            scores_chunk(3, 1)
            st_w(1)
            av_chunk(3, 0)
            st_w(2)
            av_chunk(3, 1)
            st_w(3)

    nc.finalize()
    return nc


def _host_inputs(x, freqs_cos, freqs_sin, wq, wk, wv, wo):
    """Build the 8 per-core input maps (all host-side preprocessing)."""
    x = np.asarray(x, np.float32)
    cos = np.asarray(freqs_cos, np.float32)  # [S, 32]
    sin = np.asarray(freqs_sin, np.float32)
    wq = np.asarray(wq, np.float32)
    wk = np.asarray(wk, np.float32)
    wv = np.asarray(wv, np.float32)
    wo = np.asarray(wo, np.float32)

    perm = np.concatenate([np.arange(0, HD, 2), np.arange(1, HD, 2)])  # de-interleave

    def dr_pairs(a):
        # [D, F] -> [8*128, 2*F] DoubleRow pair layout
        f = a.shape[1]
        return np.ascontiguousarray(
            a.reshape(NKP, 2, P, f).transpose(0, 2, 1, 3).reshape(NKP * P, 2 * f)
        )

    xt = np.asarray(x[0].T, dtype=BF16).astype(np.float32)  # [D, S]
    x_hi = xt.astype(F8)
    x_lo = (xt - x_hi.astype(np.float32)).astype(F8)
    xhi_p = dr_pairs(x_hi.astype(np.float32)).astype(F8)
    xlo_p = dr_pairs(x_lo.astype(np.float32)).astype(F8)

    # rope tables, stacked for two heads
    cos128 = np.empty((P, S), np.float16)
    swap128 = np.empty((P, S), np.float16)
    for dd in range(P):
        i = dd % 32
        r = dd % HD
        cos128[dd] = cos[:, i].astype(np.float16)
        swap128[dd] = (sin[:, i] if r < 32 else -sin[:, i]).astype(np.float16)

    pp = np.arange(P)[:, None]
    ff = np.arange(P)[None, :]
    masks2 = np.tile((pp <= ff).astype(np.float32), (1, 2)).astype(BF16)

    ident = np.eye(HD, dtype=np.float32).astype(BF16)

    in_maps = []
    for c in range(NCORES):
        wq_c = wq[c * QCOLS : (c + 1) * QCOLS].reshape(QH, HD, D)[:, perm, :].reshape(
            QCOLS, D
        )
        wk_c = wk[c * HD : (c + 1) * HD][perm, :]
        wv_c = wv[c * HD : (c + 1) * HD]
        wqkvt = np.concatenate([wq_c, wk_c, wv_c], axis=0).T * WS  # [D, 384]
        w_hi = wqkvt.astype(F8)
        w_lo = (wqkvt - w_hi.astype(np.float32)).astype(F8)
        whi_p = dr_pairs(w_hi.astype(np.float32)).astype(F8)
        wlo_p = dr_pairs(w_lo.astype(np.float32)).astype(F8)
        wot = np.ascontiguousarray(wo[:, c * QCOLS : (c + 1) * QCOLS].T).astype(BF16)
        in_maps.append(
            {
                "xhi": xhi_p,
                "xlo": xlo_p,
                "whi": whi_p,
                "wlo": wlo_p,
                "wot": wot,
                "cos128": cos128,
                "swap128": swap128,
                "masks2": masks2,
                "ident": ident,
            }
        )
    return in_maps


def kernel(x, freqs_cos, freqs_sin, wq, wk, wv, wo):
    from concourse.bass_utils import run_bass_kernel_spmd

    if "nc" not in _CACHE:
        _CACHE["nc"] = _build()
    nc = _CACHE["nc"]
    in_maps = _host_inputs(x, freqs_cos, freqs_sin, wq, wk, wv, wo)
    res = run_bass_kernel_spmd(nc, in_maps, core_ids=list(range(NCORES)))
    out = np.zeros((S, D), np.float64)
    for r in res.results:
        out += r["out"].astype(np.float64)
    return out.astype(np.float32).reshape(1, S, D)


# revision 33
# speedup vs baseline: 1.2978x; 1.0148x over previous
"""GQA causal attention (llama3-style RoPE) on 8 TRN2 NeuronCores.

Sharding: tensor-parallel over heads. Core c gets q-heads 4c..4c+3 and
kv-head c (GQA groups intact), plus the matching row-block of wo.T.
Each core computes a full [S, D] partial of the output projection;
the host sums the 8 partials (the "all-reduce" of the row-sharded wo).

v3 per-core pipeline, fully chunk-interleaved so ACT (exp) starts ~10us
into the kernel and PE never waits long:
  for each 512-col seq chunk j:
    qkvT[col, js] = wqkvT.T @ xT    fp8e4m3 hi/lo 3-term DoubleRow
                                    (weights host-scaled x256; descale
                                    folded into the exp scale / v drain)
    RoPE on qT/kT                   two q heads stacked per [128, 512]
    v -> vaug[sk, hd|1]             PE transpose, Pool drain
    sT[sk, 2, sq] = kT.T @ qT       K=64 row-groups: head-even rows
                                    0:64, head-odd rows 64:128 (kt dup)
    eT = exp(sT/(8*WS^2) - 3)       one ACT op per [128, 2, 512-off]
    av[sq, 2*(hd|den)] = et.T @ vaug   et slabs stationary (LdWeights
                                    free), 65-wide moving operand
    y2 = av/den                     DVE tensor_scalar divide
    yt = dma_transpose(y2)          SP queue, xbar 14ns/tile
    out[sq, d] = yT.T @ woT         bf16 partial out; host sums cores
"""

import sys

for _p in ("/opt/trn_rl_repo", "/root/.axon_site/_ro/trn_rl_repo"):
    if _p not in sys.path:
        sys.path.insert(0, _p)

import numpy as np
import ml_dtypes

import concourse.bacc as bacc
import concourse.mybir as mybir
import concourse.tile as tile

BF16 = ml_dtypes.bfloat16
F8 = ml_dtypes.float8_e4m3

S = 2048
D = 2048
HD = 64
NH = 32
NKV = 8
NCORES = 8
QH = NH // NCORES            # 4 local q heads
QCOLS = QH * HD              # 256
P = 128
NK = D // P                  # 16 contraction tiles
NKP = NK // 2                # 8 DoubleRow pairs
NSQ = S // P                 # 16 seq tiles of 128
NCH = 4                      # seq chunks of 512
CH = 512
WS = 256.0                   # fp8 weight pre-scale
CSH = 3.0                    # exp shift (cancels in softmax ratio)

_CACHE = {}


def _build():
    bf = mybir.dt.bfloat16
    f16 = mybir.dt.float16
    f32 = mybir.dt.float32
    f8 = mybir.dt.float8e4
    DR = mybir.MatmulPerfMode.DoubleRow
    Exp = mybir.ActivationFunctionType.Exp

    nc = bacc.Bacc()
    xhi_d = nc.dram_tensor("xhi", [NKP * P, 2 * S], f8, kind="ExternalInput")
    xlo_d = nc.dram_tensor("xlo", [NKP * P, 2 * S], f8, kind="ExternalInput")
    whi_d = nc.dram_tensor("whi", [NKP * P, 2 * 384], f8, kind="ExternalInput")
    wlo_d = nc.dram_tensor("wlo", [NKP * P, 2 * 384], f8, kind="ExternalInput")
    wothi_d = nc.dram_tensor("wothi", [P, 2 * D], f8, kind="ExternalInput")
    wotlo_d = nc.dram_tensor("wotlo", [P, 2 * D], f8, kind="ExternalInput")
    cos_d = nc.dram_tensor("cos128", [P, S], f16, kind="ExternalInput")
    swap_d = nc.dram_tensor("swap128", [P, S], f16, kind="ExternalInput")
    masks_d = nc.dram_tensor("masks2", [P, 2 * P], bf, kind="ExternalInput")
    ident_d = nc.dram_tensor("ident", [HD, HD], bf, kind="ExternalInput")
    out_d = nc.dram_tensor("out", [S, D], bf, kind="ExternalOutput")

    with tile.TileContext(nc) as tc:
        with (
            tc.tile_pool(name="const", bufs=1) as cpool,
            tc.tile_pool(name="xw", bufs=16) as xwpool,
            tc.tile_pool(name="vaug", bufs=16) as vpool,
            tc.tile_pool(name="et", bufs=33) as epool,
            tc.tile_pool(name="rope", bufs=2) as rpool,
            tc.tile_pool(name="y2", bufs=4) as ypool,
            tc.tile_pool(name="ot", bufs=4) as opool,
            tc.tile_pool(name="ps_a", bufs=2, space="PSUM") as ps_a,
            tc.tile_pool(name="ps_s", bufs=2, space="PSUM") as ps_s,
            tc.tile_pool(name="ps_av", bufs=2, space="PSUM") as ps_av,
        ):
            # ---- constants + weights + x in ------------------------------
            cos_sb = cpool.tile([P, S], f16, tag="cos")
            swap_sb = cpool.tile([P, S], f16, tag="swap")
            masks_sb = cpool.tile([P, 2, P], bf, tag="masks")
            ident_sb = cpool.tile([HD, HD], bf, tag="ident")
            cbias = cpool.tile([P, 1], f32, tag="cbias")
            nc.gpsimd.memset(cbias[:], -CSH)
            nc.gpsimd.dma_start(masks_sb[:], masks_d[:])
            nc.gpsimd.dma_start(ident_sb[:], ident_d[:])

            whi_sb, wlo_sb = [], []
            for kp in range(NKP):
                for lst, dram, nm in ((whi_sb, whi_d, "whi"), (wlo_sb, wlo_d, "wlo")):
                    w = xwpool.tile([P, 2, 384], f8, tag="w", name=f"{nm}{kp}")
                    eng = [nc.sync, nc.scalar][kp % 2]
                    eng.dma_start(w[:], dram[kp * P : (kp + 1) * P, :])
                    lst.append(w)

            xhi_sb, xlo_sb = [], []
            for kp in range(NKP):
                xhi_sb.append(xwpool.tile([P, 2, S], f8, tag="x", name=f"xhi{kp}"))
                xlo_sb.append(xwpool.tile([P, 2, S], f8, tag="x", name=f"xlo{kp}"))
            # x loads in 3 col-groups: [0:512), [512:1024), [1024:2048)
            for gi, (g0, g1) in enumerate(((0, CH), (CH, 2 * CH), (2 * CH, S))):
                for kp in range(NKP):
                    for xi, (sb_t, dram) in enumerate(
                        ((xhi_sb, xhi_d), (xlo_sb, xlo_d))
                    ):
                        src3 = dram[kp * P : (kp + 1) * P, :].rearrange(
                            "p (two s) -> p two s", two=2
                        )
                        eng = (
                            [nc.sync, nc.scalar][(kp + xi) % 2]
                            if gi < 2
                            else [nc.sync, nc.gpsimd][(kp + xi) % 2]
                        )
                        eng.dma_start(sb_t[kp][:, :, g0:g1], src3[:, :, g0:g1])

            nc.scalar.dma_start(cos_sb[:], cos_d[:])
            nc.scalar.dma_start(swap_sb[:], swap_d[:])
            wothi_sb = cpool.tile([P, 2, D], f8, tag="wothi")
            wotlo_sb = cpool.tile([P, 2, D], f8, tag="wotlo")
            nc.scalar.dma_start(wothi_sb[:], wothi_d[:])
            nc.scalar.dma_start(wotlo_sb[:], wotlo_d[:])

            qt_sb = [cpool.tile([P, S], f16, tag=f"qt{m}", name=f"qt{m}") for m in range(2)]
            kt_sb = cpool.tile([P, S], f16, tag="kt")
            vt_sb = cpool.tile([HD, S], bf, tag="vt")
            yt_sb = [cpool.tile([P, S], bf, tag=f"yt{m}", name=f"yt{m}") for m in range(2)]
            yt8hi_sb = cpool.tile([P, 2, S], f8, tag="yt8hi")
            yt8lo_sb = cpool.tile([P, 2, S], f8, tag="yt8lo")
            vaug_sb = [None] * NSQ

            # ---- helpers -------------------------------------------------
            def proj_psum(m, j):
                js = slice(j * CH, (j + 1) * CH)
                ps = ps_a.tile([P, CH], f32, tag="proj", name="ps_proj")
                with nc.named_scope("proj"):
                    n = 0
                    for kp in range(NKP):
                        for wt, xt in (
                            (whi_sb[kp], xhi_sb[kp]),
                            (whi_sb[kp], xlo_sb[kp]),
                            (wlo_sb[kp], xhi_sb[kp]),
                        ):
                            nc.tensor.matmul(
                                ps[:],
                                wt[:, :, m * P : (m + 1) * P],
                                xt[:, :, js],
                                start=(n == 0),
                                stop=(n == 3 * NKP - 1),
                                perf_mode=DR,
                            )
                            n += 1
                return ps

            def rope128(dst, ps, j):
                # two heads stacked: rows 0:64 head-even, 64:128 head-odd
                js = slice(j * CH, (j + 1) * CH)
                with nc.named_scope("rope"):
                    qr = rpool.tile([P, CH], f16, tag="qr", name="qr")
                    nc.vector.tensor_copy(qr[:], ps[:])
                    t2 = rpool.tile([P, CH], f16, tag="t2", name="t2")
                    for b in range(4):
                        d0 = b * 32
                        s0 = (b ^ 1) * 32
                        eng = nc.vector if b < 2 else nc.gpsimd
                        eng.tensor_mul(
                            t2[d0 : d0 + 32, :], qr[s0 : s0 + 32, :], swap_sb[s0 : s0 + 32, js]
                        )
                    nc.vector.tensor_mul(dst[:, js], qr[:], cos_sb[:, js])
                    nc.vector.tensor_add(dst[:, js], dst[:, js], t2[:])

            def rope_kv(ps, j):
                js = slice(j * CH, (j + 1) * CH)
                with nc.named_scope("rope"):
                    qr = rpool.tile([P, CH], f16, tag="qr", name="qr_k")
                    nc.vector.tensor_copy(qr[0:HD, :], ps[0:HD, :])
                    # v drain first: frees the proj psum before the rope muls
                    nc.vector.tensor_scalar_mul(vt_sb[:, js], ps[HD:P, :], 1.0 / WS)
                    t2 = rpool.tile([P, CH], f16, tag="t2", name="t2_k")
                    for b in range(2):
                        d0 = b * 32
                        s0 = (b ^ 1) * 32
                        nc.vector.tensor_mul(
                            t2[d0 : d0 + 32, :], qr[s0 : s0 + 32, :], swap_sb[s0 : s0 + 32, js]
                        )
                    nc.vector.tensor_mul(kt_sb[0:HD, js], qr[0:HD, :], cos_sb[0:HD, js])
                    nc.vector.tensor_add(kt_sb[0:HD, js], kt_sb[0:HD, js], t2[0:HD, :])
                # duplicate k rows for the odd-head row group
                nc.gpsimd.dma_start(kt_sb[HD:P, js], kt_sb[0:HD, js])

            def vtrans(j):
                with nc.named_scope("vtrans"):
                    for i in range(4 * j, 4 * j + 4):
                        pt = ps_av.tile([P, HD], bf, tag="av", name="ps_vt")
                        nc.tensor.transpose(pt[:], vt_sb[:, i * P : (i + 1) * P], ident_sb[:])
                        va = vpool.tile([P, HD + 1], bf, tag="vaug", name=f"vaug{i}")
                        nc.vector.tensor_copy(va[:, 0:HD], pt[:])
                        nc.gpsimd.memset(va[:, HD : HD + 1], 1.0)
                        vaug_sb[i] = va

            all_ets = {}

            def scores_chunk(j, m):
                nlive = 4 * j + 4
                ets = []
                with nc.named_scope("scores"):
                    for i in range(nlive):
                        off = max(0, i - 4 * j) * P
                        ps2 = ps_s.tile([P, 2, CH], f32, tag="sc", name="ps_sc")
                        for u in range(2):
                            rg = slice(u * HD, (u + 1) * HD)
                            nc.tensor.matmul(
                                ps2[:, u, off:],
                                kt_sb[rg, i * P : (i + 1) * P],
                                qt_sb[m][rg, j * CH + off : (j + 1) * CH],
                                start=True,
                                stop=True,
                            )
                        et = epool.tile([P, 2, CH], bf, tag="et", name="et")
                        with nc.named_scope("exp"):
                            nc.scalar.activation(
                                et[:, :, off:],
                                ps2[:, :, off:],
                                Exp,
                                bias=cbias[:],
                                scale=0.125 / (WS * WS),
                            )
                        if i >= 4 * j:
                            with nc.named_scope("mask"):
                                nc.gpsimd.tensor_mul(
                                    et[:, :, off : off + P],
                                    et[:, :, off : off + P],
                                    masks_sb[:],
                                )
                        ets.append(et)
                all_ets[(j, m)] = ets

            def av_chunk(j, m, rlist=(0, 1, 2, 3)):
                ets = all_ets[(j, m)]
                for r in rlist:
                    t = 4 * j + r
                    pav = ps_av.tile([P, 2 * (HD + 1)], f32, tag="av", name="ps_av")
                    with nc.named_scope("av"):
                        for u in range(2):
                            for i in range(t + 1):
                                nc.tensor.matmul(
                                    pav[:, u * (HD + 1) : (u + 1) * (HD + 1)],
                                    ets[i][:, u, r * P : (r + 1) * P],
                                    vaug_sb[i][:],
                                    start=(i == 0),
                                    stop=(i == t),
                                )
                    with nc.named_scope("norm"):
                        y2 = ypool.tile([P, P], bf, tag="y2", name="y2")
                        rc = ypool.tile([P, 2], f32, tag="rc", name="rc")
                        nc.vector.reciprocal(
                            rc[:], pav[:, HD : 2 * HD + 2 : HD + 1]
                        )
                        for u in range(2):
                            c0 = u * (HD + 1)
                            nc.vector.tensor_scalar(
                                y2[:, u * HD : (u + 1) * HD],
                                pav[:, c0 : c0 + HD],
                                rc[:, u : u + 1],
                                None,
                                mybir.AluOpType.mult,
                            )
                        nc.sync.dma_start_transpose(
                            yt_sb[m][:, t * P : (t + 1) * P], y2[:]
                        )
                        # y -> fp8 hi/lo on Pool (SBUF-only, legal there)
                        tb = slice(t * P, (t + 1) * P)
                        nc.gpsimd.tensor_copy(yt8hi_sb[:, m, tb], yt_sb[m][:, tb])
                        nc.gpsimd.tensor_sub(
                            yt8lo_sb[:, m, tb], yt_sb[m][:, tb], yt8hi_sb[:, m, tb]
                        )

            def wo_sm(sm, n):
                srow = slice(sm * P, (sm + 1) * P)
                for half in range(2):
                    ot = opool.tile([P, 2 * CH], bf, tag="ot", name="ot")
                    for q in range(2):
                        dcJ = 2 * half + q
                        dch = slice(dcJ * CH, (dcJ + 1) * CH)
                        pw = ps_a.tile([P, CH], f32, tag="proj", name="ps_wo")
                        with nc.named_scope("wo"):
                            for ti, (yw, ww) in enumerate(
                                (
                                    (yt8hi_sb, wothi_sb),
                                    (yt8lo_sb, wothi_sb),
                                    (yt8hi_sb, wotlo_sb),
                                )
                            ):
                                nc.tensor.matmul(
                                    pw[:],
                                    yw[:, :, srow],
                                    ww[:, :, dch],
                                    start=(ti == 0),
                                    stop=(ti == 2),
                                    perf_mode=DR,
                                )
                        with nc.named_scope("outdma"):
                            # GPSIMD cannot read PSUM: drains go DVE, with
                            # the post-exp last chunk split DVE/ACT
                            if sm >= 12 and (2 * n + q) % 2 == 0:
                                nc.scalar.copy(ot[:, q * CH : (q + 1) * CH], pw[:])
                            else:
                                nc.vector.tensor_copy(ot[:, q * CH : (q + 1) * CH], pw[:])
                    with nc.named_scope("outdma"):
                        nc.sync.dma_start(
                            out_d[srow, half * 2 * CH : (half + 1) * 2 * CH],
                            ot[:],
                        )

            # ---- software-pipelined main loop ----------------------------
            # P(j) proj+rope | S(j) scores+exp | V(j) av+norm | W(j) wo.
            # Interleaved so PE streams P/W work while ACT runs exp ahead.
            def st_p(j):
                rope_kv(proj_psum(2, j), j)
                rope128(qt_sb[0], proj_psum(0, j), j)
                rope128(qt_sb[1], proj_psum(1, j), j)
                vtrans(j)

            def st_s(j):
                scores_chunk(j, 0)
                scores_chunk(j, 1)

            def st_v(j):
                av_chunk(j, 0)
                av_chunk(j, 1)

            def st_w(j):
                for sm in range(4 * j, 4 * j + 4):
                    wo_sm(sm, sm)

            st_p(0)
            st_p(1)
            st_s(0)
            st_p(2)
            st_v(0)
            st_s(1)
            st_p(3)
            st_v(1)
            st_s(2)
            st_v(2)
            scores_chunk(3, 0)
            st_w(0)
            scores_chunk(3, 1)
            st_w(1)
            av_chunk(3, 0)
            st_w(2)
            for r in range(4):
                av_chunk(3, 1, (r,))
                wo_sm(12 + r, 12 + r)

    nc.finalize()
    return nc


def _host_inputs(x, freqs_cos, freqs_sin, wq, wk, wv, wo):
    """Build the 8 per-core input maps (all host-side preprocessing)."""
    x = np.asarray(x, np.float32)
    cos = np.asarray(freqs_cos, np.float32)  # [S, 32]
    sin = np.asarray(freqs_sin, np.float32)
    wq = np.asarray(wq, np.float32)
    wk = np.asarray(wk, np.float32)
    wv = np.asarray(wv, np.float32)
    wo = np.asarray(wo, np.float32)

    perm = np.concatenate([np.arange(0, HD, 2), np.arange(1, HD, 2)])  # de-interleave

    def dr_pairs(a):
        # [D, F] -> [8*128, 2*F] DoubleRow pair layout
        f = a.shape[1]
        return np.ascontiguousarray(
            a.reshape(NKP, 2, P, f).transpose(0, 2, 1, 3).reshape(NKP * P, 2 * f)
        )

    xt = np.asarray(x[0].T, dtype=BF16).astype(np.float32)  # [D, S]
    x_hi = xt.astype(F8)
    x_lo = (xt - x_hi.astype(np.float32)).astype(F8)
    xhi_p = dr_pairs(x_hi.astype(np.float32)).astype(F8)
    xlo_p = dr_pairs(x_lo.astype(np.float32)).astype(F8)

    # rope tables, stacked for two heads
    cos128 = np.empty((P, S), np.float16)
    swap128 = np.empty((P, S), np.float16)
    for dd in range(P):
        i = dd % 32
        r = dd % HD
        cos128[dd] = cos[:, i].astype(np.float16)
        swap128[dd] = (sin[:, i] if r < 32 else -sin[:, i]).astype(np.float16)

    pp = np.arange(P)[:, None]
    ff = np.arange(P)[None, :]
    masks2 = np.tile((pp <= ff).astype(np.float32), (1, 2)).astype(BF16)

    ident = np.eye(HD, dtype=np.float32).astype(BF16)

    in_maps = []
    for c in range(NCORES):
        wq_c = wq[c * QCOLS : (c + 1) * QCOLS].reshape(QH, HD, D)[:, perm, :].reshape(
            QCOLS, D
        )
        wk_c = wk[c * HD : (c + 1) * HD][perm, :]
        wv_c = wv[c * HD : (c + 1) * HD]
        wqkvt = np.concatenate([wq_c, wk_c, wv_c], axis=0).T * WS  # [D, 384]
        w_hi = wqkvt.astype(F8)
        w_lo = (wqkvt - w_hi.astype(np.float32)).astype(F8)
        whi_p = dr_pairs(w_hi.astype(np.float32)).astype(F8)
        wlo_p = dr_pairs(w_lo.astype(np.float32)).astype(F8)
        wot = np.ascontiguousarray(wo[:, c * QCOLS : (c + 1) * QCOLS].T) * WS
        wot_hi = wot.astype(F8)
        wot_lo = (wot - wot_hi.astype(np.float32)).astype(F8)

        def wot_pairs(a):
            # [256, D] -> [128, 2*D]: row p holds (row p | row 128+p)
            return np.ascontiguousarray(
                a.reshape(2, P, D).transpose(1, 0, 2).reshape(P, 2 * D)
            )

        wothi_p = wot_pairs(wot_hi.astype(np.float32)).astype(F8)
        wotlo_p = wot_pairs(wot_lo.astype(np.float32)).astype(F8)
        in_maps.append(
            {
                "xhi": xhi_p,
                "xlo": xlo_p,
                "whi": whi_p,
                "wlo": wlo_p,
                "wothi": wothi_p,
                "wotlo": wotlo_p,
                "cos128": cos128,
                "swap128": swap128,
                "masks2": masks2,
                "ident": ident,
            }
        )
    return in_maps


def kernel(x, freqs_cos, freqs_sin, wq, wk, wv, wo):
    from concourse.bass_utils import run_bass_kernel_spmd

    if "nc" not in _CACHE:
        _CACHE["nc"] = _build()
    nc = _CACHE["nc"]
    in_maps = _host_inputs(x, freqs_cos, freqs_sin, wq, wk, wv, wo)
    res = run_bass_kernel_spmd(nc, in_maps, core_ids=list(range(NCORES)))
    out = np.zeros((S, D), np.float64)
    for r in res.results:
        out += r["out"].astype(np.float64)
    return (out / WS).astype(np.float32).reshape(1, S, D)
